# revision 28
# baseline (speedup 1.0000x reference)
"""Adaptive frequency reassemble kernel for 8 TRN2 NeuronCores.

Sharding: pure data parallel over (B, D): core i owns batch b=i//4 and
d-slab [8*(i%4), 8*(i%4)+8) -> 32768 positions/core.  x_lf / x_hf are
stacked into one [128, 32768] tensor per core (lf channels on partitions
0-63, hf on 64-127).

Algebraic folds (host, exact):
  tok_t  = tokens @ W_t2f.T + b_t2f
  M      = (tok_t @ W_delta.T) * scale
  G      = M @ W_gate.T                  [8, 64]
  bg2    = W_gate @ (b_delta*scale) + b_gate
so   gate = sigmoid(G.T @ softmax_weights + bg2), and
  base   = Wsel.T @ xs  with Wsel = [diag(w_lf); diag(w_hf)]  (one matmul).

The SE-gate context (global per-(b,channel) mean) is computed ON DEVICE:
per-tile row-sums on the GPSIMD engine fused with the f32->bf16 convert,
then a tiny [128] AllReduce across the 4 cores sharing each batch, then
the 2-layer gate MLP on device.  Phase B recomputes gate from the cached
normalized attention weights (en_all, bf16 resident) and the resident
bf16 copy of xs, so each input byte is DMA'd exactly once.

Attention sub-tiles are packed 2x at PE quadrant offsets {0,32} so
exp/reciprocal/normalize run on half the free-dim width.
"""

import sys

import numpy as np

if "/opt/trn_rl_repo" not in sys.path:
    sys.path.insert(0, "/opt/trn_rl_repo")

_B, _C, _D, _H, _W = 2, 64, 32, 64, 64
_K = 8
_NCORES = 8
_NPOS = (_B * _D // _NCORES) * _H * _W  # 32768 positions per core
_NT = 1024  # phase A tile width
_NS = 512   # sub-tile width (pack factor 2)
_NTB = 1024  # phase B tile width

_NC_CACHE = {}


def _build_nc():
    import concourse.bass as bass
    import concourse.bacc as bacc
    import concourse.mybir as mybir
    from concourse import tile
    from concourse.alu_op_type import AluOpType

    f32 = mybir.dt.float32
    bf16 = mybir.dt.bfloat16
    AF = mybir.ActivationFunctionType

    nc = bacc.Bacc(None, num_devices=1 if no_cc else _NCORES)

    xs_d = nc.declare_dram_parameter("xs", [128, _NPOS], f32, isOutput=False)
    tok_d = nc.declare_dram_parameter("tok32", [128, 32], f32, isOutput=False)
    b2_d = nc.declare_dram_parameter("B2", [64, 2], f32, isOutput=False)
    bt2_d = nc.declare_dram_parameter("Bt2", [2, 64], f32, isOutput=False)
    g2_d = nc.declare_dram_parameter("G2", [64, 64], f32, isOutput=False)
    bg2_d = nc.declare_dram_parameter("bg2", [128, 1], f32, isOutput=False)
    wst_d = nc.declare_dram_parameter("WsT", [128, 16], f32, isOutput=False)
    wglf_d = nc.declare_dram_parameter("WglfT", [16, 64], f32, isOutput=False)
    wghf_d = nc.declare_dram_parameter("WghfT", [16, 64], f32, isOutput=False)
    i2_d = nc.declare_dram_parameter("I2", [128, 64], f32, isOutput=False)
    out_d = nc.declare_dram_parameter("out", [64, _NPOS], f32, isOutput=True)

    cc_in = nc.dram_tensor("cc_in", [128, 1], f32)
    cc_out = nc.dram_tensor("cc_out", [128, 1], f32)
    cc_in2 = nc.dram_tensor("cc_in2", [128, 1], f32)
    cc_out2 = nc.dram_tensor("cc_out2", [128, 1], f32)

    ntiles = _NPOS // _NT  # 32
    nsub = _NT // _NS      # 2

    with tile.TileContext(nc) as tc:
        with (
            tc.tile_pool(name="const", bufs=1) as cpool,
            tc.tile_pool(name="res", bufs=1) as rpool,
            tc.tile_pool(name="sx", bufs=8) as sxpool,
            tc.tile_pool(name="work", bufs=8) as wpool,
        ):
            tok_s = cpool.tile([128, 32], f32)
            nc.sync.dma_start(tok_s[:], tok_d[:])
            b2_s = cpool.tile([64, 2], f32)
            nc.sync.dma_start(b2_s[:], b2_d[:])
            bt2_s = cpool.tile([2, 64], f32)
            nc.sync.dma_start(bt2_s[:], bt2_d[:])
            g2_s = cpool.tile([64, 64], f32)
            nc.sync.dma_start(g2_s[:], g2_d[:])
            bg2_s = cpool.tile([128, 1], f32)
            nc.sync.dma_start(bg2_s[:], bg2_d[:])
            wst_s = cpool.tile([128, 16], f32)
            nc.sync.dma_start(wst_s[:], wst_d[:])
            wglf_s = cpool.tile([16, 64], f32)
            nc.sync.dma_start(wglf_s[:], wglf_d[:])
            wghf_s = cpool.tile([16, 64], f32)
            nc.sync.dma_start(wghf_s[:], wghf_d[:])
            i2_s = cpool.tile([128, 64], f32)
            nc.sync.dma_start(i2_s[:], i2_d[:])
            # bf16 copies of all matmul weights (fp32 matmul is 4x
            # slower on the PE; bf16 runs at 1 col/cycle)
            g2bf = cpool.tile([64, 64], bf16)
            nc.vector.tensor_copy(g2bf[:], g2_s[:])
            tokbf = cpool.tile([128, 32], bf16)
            nc.vector.tensor_copy(tokbf[:], tok_s[:])
            b2bf = cpool.tile([64, 2], bf16)
            nc.vector.tensor_copy(b2bf[:], b2_s[:])
            bt2bf = cpool.tile([2, 64], bf16)
            nc.vector.tensor_copy(bt2bf[:], bt2_s[:])

            sxbf = rpool.tile([128, _NPOS], bf16)  # resident x (64 KB/part)
            # resident normalized attention weights, quadrant-packed
            en_all = rpool.tile([64, _NPOS // 2], bf16)  # 32 KB/part
            rs_cols = rpool.tile([128, ntiles], f32)     # per-tile row sums

            # ---- Phase A: stream x, attention weights + context partials ----
            psa_ctx = tc.tile_pool(name="psA", bufs=2, space="PSUM")
            with psa_ctx as psa:
                for t in range(ntiles):
                    sl = slice(t * _NT, (t + 1) * _NT)
                    sx = sxpool.tile([128, _NT], f32, tag="sx")
                    nc.sync.dma_start(sx[:], xs_d[:, sl])
                    # f32->bf16 convert fused with row-sum (context partial)
                    nc.vector.tensor_scalar(
                        sxbf[:, sl], sx[:], 1.0, 0.0, AluOpType.mult,
                        AluOpType.add, accum_out=rs_cols[:, t:t + 1],
                    )
                    # scores packed: sub-tile j -> partitions [32j, 32(j+1));
                    # token cols 8-31 are zero so rows 8-31 of each band are 0
                    psS = psa.tile([64, _NS], f32, tag="psS")
                    for j in range(nsub):
                        nc.tensor.matmul(
                            psS[32 * j:32 * (j + 1), :], tok_s[:],
                            sx[:, j * _NS:(j + 1) * _NS], start=True, stop=True,
                        )
                    E = wpool.tile([64, _NS], f32, tag="E")
                    nc.scalar.activation(E[:], psS[:], AF.Exp)
                    psD = psa.tile([2, _NS], f32, tag="psD")
                    nc.tensor.matmul(psD[:], b2_s[:], E[:], start=True, stop=True)
                    R = wpool.tile([2, _NS], f32, tag="R")
                    nc.vector.reciprocal(R[:], psD[:])
                    psRB = psa.tile([64, _NS], f32, tag="psRB")
                    nc.tensor.matmul(psRB[:], bt2_s[:], R[:], start=True, stop=True)
                    nc.vector.tensor_tensor(
                        out=en_all[:, t * _NS:(t + 1) * _NS], in0=E[:],
                        in1=psRB[:], op=AluOpType.mult,
                    )

                # ---- context AllReduce (per-batch groups) + gate MLP ----
                rs = rpool.tile([128, 1], f32)
                nc.vector.tensor_reduce(
                    rs[:], rs_cols[:], axis=mybir.AxisListType.X,
                    op=AluOpType.add,
                )
                nc.sync.dma_start(cc_in[:], rs[:])
                nc.gpsimd.collective_compute(
                    "AllReduce", AluOpType.add,
                    replica_groups=[[0, 1, 2, 3], [4, 5, 6, 7]],
                    ins=[cc_in[:]], outs=[cc_out[:]],
                )
                ctxs = rpool.tile([128, 1], f32)
                nc.sync.dma_start(ctxs[:], cc_out[:])
                # MLP psums live in the phase-A pool's two free banks
                # (fresh banks -> no WAR deps on the matmuls)
                ps1 = psa.tile([16, 1], f32, tag="psm_a", name="ps1", bufs=1)
                nc.tensor.matmul(ps1[:], wst_s[:], ctxs[:], start=True, stop=True)
                sh = rpool.tile([16, 1], f32)
                nc.scalar.activation(sh[:], ps1[:], AF.Relu)
                ps2 = psa.tile([64, 1], f32, tag="psm_b", name="ps2", bufs=1)
                nc.tensor.matmul(ps2[:], wglf_s[:], sh[:], start=True, stop=True)
                ps3 = psa.tile([64, 1], f32, tag="psm_a", name="ps3", bufs=1)
                nc.tensor.matmul(ps3[:], wghf_s[:], sh[:], start=True, stop=True)
                wvec = rpool.tile([128, 1], f32)
                nc.scalar.activation(wvec[0:64, :], ps2[:], AF.Sigmoid)
                nc.scalar.activation(wvec[64:128, :], ps3[:], AF.Sigmoid)
                # Wsel = [diag(2*sig_lf); diag(2*sig_hf)] (the *2 is baked in I2)
                wsel = rpool.tile([128, 64], bf16)
                nc.vector.tensor_scalar(
                    wsel[:], i2_s[:], wvec[:, 0:1], None, AluOpType.mult,
                )
            # ---- Phase B: base matmul + gate + combine, stream out ----
            nb = _NPOS // _NTB  # 32
            with (
                tc.tile_pool(name="psB", bufs=2, space="PSUM") as psbp,
                tc.tile_pool(name="outp", bufs=6) as opool,
            ):
                # descending order: the first iteration's waits cover all
                # later (lower) pool/DVE clock thresholds, keeping every
                # matmul within the 2-sync-wait HW limit
                for it in reversed(range(nb)):
                    psB = psbp.tile([64, _NTB], f32, tag="psB")
                    for j in range(2):
                        o0 = it * _NTB + j * _NS
                        nc.tensor.matmul(
                            psB[:, j * _NS:(j + 1) * _NS], wsel[:],
                            sxbf[:, o0:o0 + _NS], start=True, stop=True,
                        )
                    psG = psbp.tile([64, _NTB], f32, tag="psG")
                    for j in range(2):
                        q = 32 * j
                        nc.tensor.matmul(
                            psG[:, j * _NS:(j + 1) * _NS], g2bf[q:q + 8, :],
                            en_all[q:q + 8, it * _NS:(it + 1) * _NS],
                            start=True, stop=True,
                        )
                    gat = opool.tile([64, _NTB], f32, tag="gat")
                    nc.scalar.activation(
                        gat[:], psG[:], AF.Sigmoid, bias=bg2_s[:, 0:1],
                    )
                    outt = opool.tile([64, _NTB], f32, tag="outt")
                    nc.vector.scalar_tensor_tensor(
                        outt[:], gat[:], 1.0, psB[:],
                        AluOpType.add, AluOpType.mult,
                    )
                    nc.sync.dma_start(
                        out_d[:, it * _NTB:(it + 1) * _NTB], outt[:],
                    )

    nc.compile()
    nc.finalize()
    return nc


def _get_nc(repeat=1, no_cc=False):
    key = f"nc{repeat}_{no_cc}"
    if key not in _NC_CACHE:
        _NC_CACHE[key] = _build_nc(repeat, no_cc)
    return _NC_CACHE[key]


def _host_params(inputs):
    f = np.float32
    tokens = np.asarray(inputs["tokens"], f)
    scale = float(np.asarray(inputs["scale"]).reshape(-1)[0])
    sf = _C ** -0.5
    tok32 = np.zeros((128, 32), f)
    tok32[0:64, 0:_K] = tokens.T * sf
    tok32[64:128, 0:_K] = tokens.T * sf
    tok_t = tokens @ np.asarray(inputs["W_t2f"], f).T + np.asarray(inputs["b_t2f"], f)
    M = (tok_t @ np.asarray(inputs["W_delta"], f).T) * scale
    W_gate = np.asarray(inputs["W_gate"], f)
    G = M @ W_gate.T  # [8, 64]
    bg2v = (W_gate @ (np.asarray(inputs["b_delta"], f) * scale)
            + np.asarray(inputs["b_gate"], f))
    bg2 = np.concatenate([bg2v, bg2v])[:, None]  # stacked for both halves
    # quadrant-packed selector / replication matrices (bands at 0 and 32)
    B2 = np.zeros((64, 2), f)
    Bt2 = np.zeros((2, 64), f)
    G2 = np.zeros((64, 64), f)
    for j in range(2):
        B2[32 * j:32 * j + 8, j] = 1.0
        Bt2[j, 32 * j:32 * j + 8] = 1.0
        G2[32 * j:32 * j + 8, :] = G
    WsT = np.ascontiguousarray(
        np.asarray(inputs["W_shared"], f).T / (_D * _H * _W))
    WglfT = np.ascontiguousarray(np.asarray(inputs["W_glf"], f).T)
    WghfT = np.ascontiguousarray(np.asarray(inputs["W_ghf"], f).T)
    eye2 = np.eye(64, dtype=f) * 2.0
    I2 = np.ascontiguousarray(np.concatenate([eye2, eye2], 0))
    return {
        "tok32": tok32, "B2": B2, "Bt2": Bt2, "G2": G2,
        "bg2": np.ascontiguousarray(bg2, f), "WsT": WsT,
        "WglfT": WglfT, "WghfT": WghfT, "I2": I2,
    }


def kernel(**inputs):
    from concourse.bass_utils import run_bass_kernel_spmd

    x_hf = np.asarray(inputs["x_hf"], np.float32)
    x_lf = np.asarray(inputs["x_lf"], np.float32)
    params = _host_params(inputs)

    in_maps = []
    for i in range(_NCORES):
        b, d0 = i // 4, 8 * (i % 4)
        xl = x_lf[b, :, d0:d0 + 8].reshape(64, -1)
        xh = x_hf[b, :, d0:d0 + 8].reshape(64, -1)
        xs = np.ascontiguousarray(np.concatenate([xl, xh], 0), np.float32)
        m = {"xs": xs}
        m.update(params)
        in_maps.append(m)

    nc = _get_nc()
    res = run_bass_kernel_spmd(nc, in_maps, list(range(_NCORES)))
    out = np.empty((_B, _C, _D, _H, _W), np.float32)
    for i in range(_NCORES):
        b, d0 = i // 4, 8 * (i % 4)
        out[b, :, d0:d0 + 8] = np.asarray(res.results[i]["out"]).reshape(
            64, 8, _H, _W)
    return out
